# revision 19
# baseline (speedup 1.0000x reference)
"""Additive (Bahdanau) attention fused Trainium2 kernel.

Strategy
--------
The reference materializes a [B, Lq, Lk, D] = 768MB broadcast intermediate:
    scores[q,k] = sum_d w_d * tanh(Q[q,d] + K[k,d]) + b_att
We never materialize it.  tanh(q+k) is approximated by a truncated Fourier
sine series P(x) = sum_m c_m sin(omega_m x) fit on [-5.2, 5.2]; the angle
addition formula makes each term separable:
    sin(w(q+k)) = sin(wq)cos(wk) + cos(wq)sin(wk)
so scores = A @ B^T with A = [per-q sin/cos basis * c_m * w_d] (bf16) and
B = [per-k cos/sin basis] (bf16), contracting over (m, trig, d) = 2*M*768 on
the TensorEngine.  Basis tensors are built with a magic-number range
reduction on the VectorEngine (ACT's Sin is only valid on |x| <~ 3.2):
    tau = x * omega/2pi + (768.0 + phase_turns)   # fp32, ulp = 2^-14
    w14 = lowbits14(bitpattern(tau))              # frac(turns) * 16384
    basis = Sin(w14 * 2pi/16384 - pi)             # = -sin(omega x + phase)
The global -1 appears on BOTH sides of every product, so it cancels.

The final +Q output term reuses the already-computed Q^T (which carries
bq+bk) via accumulating PE transposes; the bias row compensates with
bt - bk.

Sharding: sequence-parallel over the query axis -- each of the 8 cores owns
L/8 = 64 queries; hidden_states / weights / K are replicated.  Per-core
output slab [64, 768] is concatenated on the host.
"""

import os
import sys

for _p in ("/opt/trn_rl_repo",):
    if _p not in sys.path:
        sys.path.insert(0, _p)

import numpy as np
import ml_dtypes

import concourse.bacc as bacc
import concourse.tile as tile
from concourse import mybir
from concourse.bass_utils import run_bass_kernel_spmd

AF = mybir.ActivationFunctionType
ALU = mybir.AluOpType
F32 = mybir.dt.float32
BF16 = mybir.dt.bfloat16
I32 = mybir.dt.int32
NPBF16 = ml_dtypes.bfloat16

B, L, D = 1, 512, 768
CORES = 8
QL = L // CORES          # 64 queries per core
DC = D // 128            # 6 chunks of 128 along d
KC = L // 128            # 4 chunks of 128 along k

M_HARM = 5
PERIOD = 6.0
FIT_RANGE = 5.2
TWO_PI = float(2 * np.pi)
MAGIC = 768.0            # 1.5 * 2^9 -> fp32 ulp 2^-14 for values near 768
NBITS = 14
SIN_SCALE = TWO_PI / (1 << NBITS)


def _fit_coefficients():
    om = np.pi * np.arange(1, M_HARM + 1) / PERIOD
    g = np.linspace(-FIT_RANGE, FIT_RANGE, 8001)
    A = np.sin(np.outer(g, om))
    coef, *_ = np.linalg.lstsq(A, np.tanh(g), rcond=None)
    return om.astype(np.float64), coef.astype(np.float64)

OMEGAS, COEFS = _fit_coefficients()

_NC = None


def _build():
    nc = bacc.Bacc("TRN2", target_bir_lowering=False, debug=False)

    dr = {}
    # critical-path inputs first (QT/KT + Q basis), bulk epilogue inputs last
    dr["hsTloc"] = nc.dram_tensor("hsTloc", [DC, 128, QL], F32, kind="ExternalInput")
    dr["Wq"] = nc.dram_tensor("Wq", [DC, 128, D], F32, kind="ExternalInput")
    dr["hsT"] = nc.dram_tensor("hsT", [DC, 128, L], BF16, kind="ExternalInput")
    dr["Wk"] = nc.dram_tensor("Wk", [DC, 128, D], BF16, kind="ExternalInput")
    dr["wcol"] = nc.dram_tensor("wcol", [128, DC * QL], F32, kind="ExternalInput")
    dr["bqk"] = nc.dram_tensor("bqk", [1, D], F32, kind="ExternalInput")
    dr["ones"] = nc.dram_tensor("ones", [1, QL], F32, kind="ExternalInput")
    dr["hs"] = nc.dram_tensor("hs", [KC, 128, D], BF16, kind="ExternalInput")
    dr["Wt"] = nc.dram_tensor("Wt", [DC, 128, D], BF16, kind="ExternalInput")
    dr["eye64"] = nc.dram_tensor("eye64", [QL, QL], BF16, kind="ExternalInput")
    dr["eye128"] = nc.dram_tensor("eye128", [128, 128], F32, kind="ExternalInput")
    dr["maskb"] = nc.dram_tensor("maskb", [QL, L], F32, kind="ExternalInput")
    dr["btk"] = nc.dram_tensor("btk", [1, D], F32, kind="ExternalInput")  # bt - bk
    out_dram = nc.dram_tensor("out", [QL, D], F32, kind="ExternalOutput")

    with tile.TileContext(nc) as tc:
        with (
            tc.tile_pool(name="big", bufs=1) as big,
            tc.tile_pool(name="qa", bufs=2) as qa_pool,
            tc.tile_pool(name="yv", bufs=2) as yv_pool,
            tc.tile_pool(name="kb", bufs=3) as kb_pool,
            tc.tile_pool(name="ps_sc", bufs=1, space="PSUM") as ps_sc,
            tc.tile_pool(name="ps_kt", bufs=2, space="PSUM") as ps_kt,
            tc.tile_pool(name="ps_sm", bufs=2, space="PSUM") as ps_sm,
        ):
            # ---- persistent SBUF tiles + input DMAs ----
            # issue split across three engines so descriptor-gen doesn't
            # serialize on one sequencer; critical path (QT/KT) first
            def load(shape, src_ap, tag, dt=F32, eng=None):
                t = big.tile(shape, dt, tag=tag)
                (eng or nc.sync).dma_start(t[:], src_ap)
                return t

            negpi = big.tile([128, 1], F32, tag="negpi")
            nc.gpsimd.memset(negpi[:], -float(np.pi))

            hsTloc_sb = [load([128, QL], dr["hsTloc"][dc], f"hsTloc{dc}") for dc in range(DC)]
            Wq_sb = [load([128, D], dr["Wq"][dc], f"Wq{dc}") for dc in range(DC)]
            hsT_sb = [load([128, L], dr["hsT"][dc], f"hsT{dc}", BF16) for dc in range(DC)]
            Wk_sb = [load([128, D], dr["Wk"][dc], f"Wk{dc}", BF16) for dc in range(DC)]
            wcol_sb = load([128, DC * QL], dr["wcol"][:], "wcol")
            bqk_sb = load([1, D], dr["bqk"][:], "bqk")
            ones_sb = load([1, QL], dr["ones"][:], "ones")
            hs_sb = [load([128, D], dr["hs"][kc], f"hs{kc}", BF16) for kc in range(KC)]
            Wt_sb = [load([128, D], dr["Wt"][dc], f"Wt{dc}", BF16) for dc in range(DC)]
            eye64_sb = load([QL, QL], dr["eye64"][:], "eye64", BF16)
            eye128_sb = load([128, 128], dr["eye128"][:], "eye128")
            maskb_sb = load([QL, L], dr["maskb"][:], "maskb")
            btk_sb = load([1, D], dr["btk"][:], "btk")

            # ---- QT = (Wq^T hsT_loc) + (bq+bk), laid out [128, DC*QL] ----
            qt_all = big.tile([128, DC * QL], F32, tag="qt_all")
            for do in range(DC):
                ps = ps_sm.tile([128, QL], F32, tag="ps_sm")
                for di in range(DC):
                    nc.tensor.matmul(
                        ps[:], Wq_sb[di][:, do * 128:(do + 1) * 128], hsTloc_sb[di][:],
                        start=(di == 0), stop=False,
                    )
                nc.tensor.matmul(
                    ps[:], bqk_sb[:, do * 128:(do + 1) * 128], ones_sb[:],
                    start=False, stop=True,
                )
                nc.vector.tensor_copy(qt_all[:, do * QL:(do + 1) * QL], ps[:])

            # ---- Q-side basis: AwT[(m,t)] [128, DC*QL] bf16 ----
            # t=0: sin(om Q) pairs with K cos; t=1: cos(om Q) pairs with K sin
            aw = {}
            for m in range(M_HARM):
                s_turn = float(OMEGAS[m] / TWO_PI)
                cm = float(COEFS[m])
                for t, phase in ((0, 0.0), (1, 0.25)):
                    yv = qa_pool.tile([128, DC * QL], F32, tag="q_yv")
                    nc.vector.tensor_scalar(
                        yv[:], qt_all[:], s_turn, MAGIC + phase, op0=ALU.mult, op1=ALU.add
                    )
                    yvi = yv[:].bitcast(I32)
                    nc.vector.tensor_scalar(
                        yvi, yvi, (1 << NBITS) - 1, None, op0=ALU.bitwise_and
                    )
                    qa = qa_pool.tile([128, DC * QL], F32, tag="q_qa")
                    nc.scalar.activation(qa[:], yvi, AF.Sin, bias=negpi[:], scale=SIN_SCALE)
                    awt = big.tile([128, DC * QL], BF16, tag=f"aw{m}_{t}")
                    nc.vector.scalar_tensor_tensor(
                        awt[:], qa[:], cm, wcol_sb[:], op0=ALU.mult, op1=ALU.mult
                    )
                    aw[(m, t)] = awt

            # ---- KT = Wk^T hsT (bf16 inputs, f32 accum), laid out [128, DC*L] ----
            kt_all = big.tile([128, DC * L], F32, tag="kt_all")
            for do in range(DC):
                ps = ps_kt.tile([128, L], F32, tag="ps_kt")
                for di in range(DC):
                    nc.tensor.matmul(
                        ps[:], Wk_sb[di][:, do * 128:(do + 1) * 128], hsT_sb[di][:],
                        start=(di == 0), stop=(di == DC - 1),
                    )
                nc.scalar.copy(kt_all[:, do * L:(do + 1) * L], ps[:])

            # ---- main: K-side basis + scores matmuls ----
            scores_ps = ps_sc.tile([QL, L], F32, tag="scores")
            n_mm = 2 * M_HARM * DC
            idx = 0
            for m in range(M_HARM):
                s_turn = float(OMEGAS[m] / TWO_PI)
                for t, phase in ((0, 0.25), (1, 0.0)):  # K side: t=0 cos, t=1 sin
                    yk = yv_pool.tile([128, DC * L], F32, tag="k_yv")
                    nc.vector.tensor_scalar(
                        yk[:], kt_all[:], s_turn, MAGIC + phase, op0=ALU.mult, op1=ALU.add
                    )
                    yki = yk[:].bitcast(I32)
                    nc.vector.tensor_scalar(
                        yki, yki, (1 << NBITS) - 1, None, op0=ALU.bitwise_and
                    )
                    kb = kb_pool.tile([128, DC * L], BF16, tag="k_kb")
                    nc.scalar.activation(kb[:], yki, AF.Sin, bias=negpi[:], scale=SIN_SCALE)
                    for dc in range(DC):
                        nc.tensor.matmul(
                            scores_ps[:],
                            aw[(m, t)][:, dc * QL:(dc + 1) * QL],
                            kb[:, dc * L:(dc + 1) * L],
                            start=(idx == 0), stop=(idx == n_mm - 1),
                        )
                        idx += 1

            # ---- softmax over k (free axis) ----
            scores_sb = big.tile([QL, L], F32, tag="scores_sb")
            nc.vector.tensor_tensor(scores_sb[:], scores_ps[:], maskb_sb[:], op=ALU.add)
            negmx = big.tile([QL, 1], F32, tag="negmx")
            nc.vector.tensor_reduce(
                negmx[:], scores_sb[:], axis=mybir.AxisListType.X, op=ALU.max, negate=True
            )
            exp_sb = big.tile([QL, L], F32, tag="exp_sb")
            nc.scalar.activation(exp_sb[:], scores_sb[:], AF.Exp, bias=negmx[:])
            sm = big.tile([QL, 1], F32, tag="sm")
            nc.vector.tensor_reduce(sm[:], exp_sb[:], axis=mybir.AxisListType.X, op=ALU.add)
            rs = big.tile([QL, 1], F32, tag="rs")
            nc.vector.reciprocal(rs[:], sm[:])
            probs = big.tile([QL, L], BF16, tag="probs")
            nc.vector.tensor_scalar(probs[:], exp_sb[:], rs[:], None, op0=ALU.mult)

            # ---- probs^T via PE transpose (bf16) ----
            probsT_sb = []
            for kc in range(KC):
                ps = ps_sm.tile([128, QL], BF16, tag="ps_pt")
                nc.tensor.matmul(
                    ps[:], probs[:, kc * 128:(kc + 1) * 128], eye64_sb[:],
                    is_transpose=True,
                )
                pt = big.tile([128, QL], BF16, tag=f"pt{kc}")
                nc.vector.tensor_copy(pt[:], ps[:])
                probsT_sb.append(pt)

            # ---- weighted^T[do] = sum_kc hs[kc,:,do-slice]^T probsT[kc] (bf16) ----
            wT_sb = []
            for do in range(DC):
                ps = ps_sm.tile([128, QL], F32, tag="ps_sm")
                for kc in range(KC):
                    nc.tensor.matmul(
                        ps[:], hs_sb[kc][:, do * 128:(do + 1) * 128], probsT_sb[kc][:],
                        start=(kc == 0), stop=(kc == KC - 1),
                    )
                wt = big.tile([128, QL], BF16, tag=f"wt{do}")
                nc.vector.tensor_copy(wt[:], ps[:])
                wT_sb.append(wt)

            # ---- out = weighted @ Wt + (bt - bk) + (Q + bq + bk) ----
            out_sb = big.tile([QL, D], F32, tag="out_sb")
            H = D // 2
            for h in range(2):
                ps = ps_sm.tile([QL, H], F32, tag="ps_sm")
                for do in range(DC):
                    nc.tensor.matmul(
                        ps[:], wT_sb[do][:], Wt_sb[do][:, h * H:(h + 1) * H],
                        start=(do == 0), stop=False,
                    )
                nc.tensor.matmul(
                    ps[:], ones_sb[:], btk_sb[:, h * H:(h + 1) * H],
                    start=False, stop=False,
                )
                for j in range(3):
                    do = h * 3 + j
                    nc.tensor.matmul(
                        ps[:, j * 128:(j + 1) * 128],
                        qt_all[:, do * QL:(do + 1) * QL],
                        eye128_sb[:],
                        is_transpose=True,
                        start=False, stop=(j == 2),
                        skip_group_check=True,
                    )
                nc.vector.tensor_copy(out_sb[:, h * H:(h + 1) * H], ps[:])

            nc.sync.dma_start(out_dram[:], out_sb[:])

    nc.compile()
    return nc


def _get_nc():
    global _NC
    if _NC is None:
        _NC = _build()
    return _NC


def kernel(hidden_states, attention_mask, Wq, bq, Wk, bk, w_att, b_att, Wt, bt):
    nc = _get_nc()

    hs = np.ascontiguousarray(np.asarray(hidden_states, dtype=np.float32)[0])  # [L, D]
    Wq = np.asarray(Wq, dtype=np.float32)
    Wk = np.asarray(Wk, dtype=np.float32)
    Wt = np.asarray(Wt, dtype=np.float32)
    bq = np.asarray(bq, dtype=np.float32)
    bk = np.asarray(bk, dtype=np.float32)
    bt = np.asarray(bt, dtype=np.float32)
    w_att = np.asarray(w_att, dtype=np.float32)
    b_att = np.float32(np.asarray(b_att))
    mask = np.asarray(attention_mask, dtype=np.float32).reshape(-1)  # [L] (B=1)

    hsT = np.ascontiguousarray(hs.T)                                  # [D, L]
    common = {
        "Wq": Wq.reshape(DC, 128, D),
        "hsT": hsT.astype(NPBF16).reshape(DC, 128, L),
        "Wk": Wk.astype(NPBF16).reshape(DC, 128, D),
        "wcol": np.ascontiguousarray(np.repeat(w_att.reshape(DC, 128).T, QL, axis=1)),  # [128, DC*QL]
        "bqk": (bq + bk).reshape(1, D),
        "ones": np.ones((1, QL), np.float32),
        "hs": hs.astype(NPBF16).reshape(KC, 128, D),
        "Wt": Wt.astype(NPBF16).reshape(DC, 128, D),
        "eye64": np.eye(QL, dtype=NPBF16),
        "eye128": np.eye(128, dtype=np.float32),
        "maskb": np.ascontiguousarray(
            np.broadcast_to(mask + b_att, (QL, L)).astype(np.float32)
        ),
        "btk": (bt - bk).reshape(1, D),
    }
    in_maps = []
    for c in range(CORES):
        m = dict(common)
        m["hsTloc"] = np.ascontiguousarray(
            hsT[:, c * QL:(c + 1) * QL].reshape(DC, 128, QL)
        )
        in_maps.append(m)

    trace = bool(int(os.environ.get("BASSK_TRACE", "0")))
    res = run_bass_kernel_spmd(nc, in_maps, core_ids=list(range(CORES)), trace=trace)
    if trace:
        kernel.last_exec_time_ns = res.exec_time_ns
        kernel.last_results = res

    out = np.concatenate([res.results[c]["out"] for c in range(CORES)], axis=0)
    return out.reshape(B, L, D).astype(np.float32)


# revision 20
# speedup vs baseline: 1.0769x; 1.0769x over previous
"""Additive (Bahdanau) attention fused Trainium2 kernel.

Strategy
--------
The reference materializes a [B, Lq, Lk, D] = 768MB broadcast intermediate:
    scores[q,k] = sum_d w_d * tanh(Q[q,d] + K[k,d]) + b_att
We never materialize it.  tanh(q+k) is approximated by a truncated Fourier
sine series P(x) = sum_m c_m sin(omega_m x) fit on [-5.2, 5.2]; the angle
addition formula makes each term separable:
    sin(w(q+k)) = sin(wq)cos(wk) + cos(wq)sin(wk)
so scores = A @ B^T with A = [per-q sin/cos basis * c_m * w_d] (bf16) and
B = [per-k cos/sin basis] (bf16), contracting over (m, trig, d) = 2*M*768 on
the TensorEngine.  Basis tensors are built with a magic-number range
reduction on the VectorEngine (ACT's Sin is only valid on |x| <~ 3.2):
    tau = x * omega/2pi + (768.0 + phase_turns)   # fp32, ulp = 2^-14
    w14 = lowbits14(bitpattern(tau))              # frac(turns) * 16384
    basis = Sin(w14 * 2pi/16384 - pi)             # = -sin(omega x + phase)
The global -1 appears on BOTH sides of every product, so it cancels.

The final +Q output term reuses the already-computed Q^T (which carries
bq+bk) via accumulating PE transposes; the bias row compensates with
bt - bk.

Sharding: sequence-parallel over the query axis -- each of the 8 cores owns
L/8 = 64 queries; hidden_states / weights / K are replicated.  Per-core
output slab [64, 768] is concatenated on the host.
"""

import os
import sys

for _p in ("/opt/trn_rl_repo",):
    if _p not in sys.path:
        sys.path.insert(0, _p)

import numpy as np
import ml_dtypes

import concourse.bacc as bacc
import concourse.tile as tile
from concourse import mybir
from concourse.bass_utils import run_bass_kernel_spmd

AF = mybir.ActivationFunctionType
ALU = mybir.AluOpType
F32 = mybir.dt.float32
BF16 = mybir.dt.bfloat16
I32 = mybir.dt.int32
NPBF16 = ml_dtypes.bfloat16

B, L, D = 1, 512, 768
CORES = 8
QL = L // CORES          # 64 queries per core
DC = D // 128            # 6 chunks of 128 along d
KC = L // 128            # 4 chunks of 128 along k

M_HARM = 5
PERIOD = 6.0
FIT_RANGE = 5.2
TWO_PI = float(2 * np.pi)
MAGIC = 768.0            # 1.5 * 2^9 -> fp32 ulp 2^-14 for values near 768
NBITS = 14
SIN_SCALE = TWO_PI / (1 << NBITS)


def _fit_coefficients():
    om = np.pi * np.arange(1, M_HARM + 1) / PERIOD
    g = np.linspace(-FIT_RANGE, FIT_RANGE, 8001)
    A = np.sin(np.outer(g, om))
    coef, *_ = np.linalg.lstsq(A, np.tanh(g), rcond=None)
    return om.astype(np.float64), coef.astype(np.float64)

OMEGAS, COEFS = _fit_coefficients()

_NC = None


def _build():
    nc = bacc.Bacc("TRN2", target_bir_lowering=False, debug=False)

    dr = {}
    # critical-path inputs first (QT/KT + Q basis), bulk epilogue inputs last
    dr["hsTloc"] = nc.dram_tensor("hsTloc", [DC, 128, QL], F32, kind="ExternalInput")
    dr["Wq"] = nc.dram_tensor("Wq", [DC, 128, D], F32, kind="ExternalInput")
    dr["hsT"] = nc.dram_tensor("hsT", [DC, 128, L], BF16, kind="ExternalInput")
    dr["Wk"] = nc.dram_tensor("Wk", [DC, 128, D], BF16, kind="ExternalInput")
    dr["wcol"] = nc.dram_tensor("wcol", [128, DC * QL], F32, kind="ExternalInput")
    dr["bqk"] = nc.dram_tensor("bqk", [1, D], F32, kind="ExternalInput")
    dr["ones"] = nc.dram_tensor("ones", [1, QL], F32, kind="ExternalInput")
    dr["hs"] = nc.dram_tensor("hs", [KC, 128, D], BF16, kind="ExternalInput")
    dr["Wt"] = nc.dram_tensor("Wt", [DC, 128, D], BF16, kind="ExternalInput")
    dr["eye64"] = nc.dram_tensor("eye64", [QL, QL], BF16, kind="ExternalInput")
    dr["eye128"] = nc.dram_tensor("eye128", [128, 128], F32, kind="ExternalInput")
    dr["maskb"] = nc.dram_tensor("maskb", [QL, L], F32, kind="ExternalInput")
    dr["btk"] = nc.dram_tensor("btk", [1, D], F32, kind="ExternalInput")  # bt - bk
    out_dram = nc.dram_tensor("out", [QL, D], F32, kind="ExternalOutput")

    with tile.TileContext(nc) as tc:
        with (
            tc.tile_pool(name="big", bufs=1) as big,
            tc.tile_pool(name="qa", bufs=2) as qa_pool,
            tc.tile_pool(name="yv", bufs=2) as yv_pool,
            tc.tile_pool(name="kb", bufs=3) as kb_pool,
            tc.tile_pool(name="ps_sc", bufs=1, space="PSUM") as ps_sc,
            tc.tile_pool(name="ps_kt", bufs=2, space="PSUM") as ps_kt,
            tc.tile_pool(name="ps_sm", bufs=2, space="PSUM") as ps_sm,
        ):
            # ---- persistent SBUF tiles + input DMAs ----
            # issue split across three engines so descriptor-gen doesn't
            # serialize on one sequencer; critical path (QT/KT) first
            def load(shape, src_ap, tag, dt=F32, eng=None):
                t = big.tile(shape, dt, tag=tag)
                (eng or nc.sync).dma_start(t[:], src_ap)
                return t

            negpi = big.tile([128, 1], F32, tag="negpi")
            nc.gpsimd.memset(negpi[:], -float(np.pi))

            hsT_sb = [load([128, L], dr["hsT"][dc], f"hsT{dc}", BF16) for dc in range(DC)]
            Wk_sb = [load([128, D], dr["Wk"][dc], f"Wk{dc}", BF16) for dc in range(DC)]
            hsTloc_sb = [load([128, QL], dr["hsTloc"][dc], f"hsTloc{dc}") for dc in range(DC)]
            Wq_sb = [load([128, D], dr["Wq"][dc], f"Wq{dc}") for dc in range(DC)]
            wcol_sb = load([128, DC * QL], dr["wcol"][:], "wcol")
            bqk_sb = load([1, D], dr["bqk"][:], "bqk")
            ones_sb = load([1, QL], dr["ones"][:], "ones")
            hs_sb = [load([128, D], dr["hs"][kc], f"hs{kc}", BF16) for kc in range(KC)]
            Wt_sb = [load([128, D], dr["Wt"][dc], f"Wt{dc}", BF16) for dc in range(DC)]
            eye64_sb = load([QL, QL], dr["eye64"][:], "eye64", BF16)
            eye128_sb = load([128, 128], dr["eye128"][:], "eye128")
            maskb_sb = load([QL, L], dr["maskb"][:], "maskb")
            btk_sb = load([1, D], dr["btk"][:], "btk")

            # ---- KT = Wk^T hsT (bf16 inputs, f32 accum), laid out [128, DC*L] ----
            kt_all = big.tile([128, DC * L], F32, tag="kt_all")
            for do in range(DC):
                ps = ps_kt.tile([128, L], F32, tag="ps_kt")
                for di in range(DC):
                    nc.tensor.matmul(
                        ps[:], Wk_sb[di][:, do * 128:(do + 1) * 128], hsT_sb[di][:],
                        start=(di == 0), stop=(di == DC - 1),
                    )
                nc.scalar.copy(kt_all[:, do * L:(do + 1) * L], ps[:])

            # ---- QT = (Wq^T hsT_loc) + (bq+bk), laid out [128, DC*QL] ----
            qt_all = big.tile([128, DC * QL], F32, tag="qt_all")
            for do in range(DC):
                ps = ps_sm.tile([128, QL], F32, tag="ps_sm")
                for di in range(DC):
                    nc.tensor.matmul(
                        ps[:], Wq_sb[di][:, do * 128:(do + 1) * 128], hsTloc_sb[di][:],
                        start=(di == 0), stop=False,
                    )
                nc.tensor.matmul(
                    ps[:], bqk_sb[:, do * 128:(do + 1) * 128], ones_sb[:],
                    start=False, stop=True,
                )
                nc.vector.tensor_copy(qt_all[:, do * QL:(do + 1) * QL], ps[:])

            # ---- Q-side basis: AwT[(m,t)] [128, DC*QL] bf16 ----
            # t=0: sin(om Q) pairs with K cos; t=1: cos(om Q) pairs with K sin
            aw = {}
            for m in range(M_HARM):
                s_turn = float(OMEGAS[m] / TWO_PI)
                cm = float(COEFS[m])
                for t, phase in ((0, 0.0), (1, 0.25)):
                    yv = qa_pool.tile([128, DC * QL], F32, tag="q_yv")
                    nc.vector.tensor_scalar(
                        yv[:], qt_all[:], s_turn, MAGIC + phase, op0=ALU.mult, op1=ALU.add
                    )
                    yvi = yv[:].bitcast(I32)
                    nc.vector.tensor_scalar(
                        yvi, yvi, (1 << NBITS) - 1, None, op0=ALU.bitwise_and
                    )
                    qa = qa_pool.tile([128, DC * QL], F32, tag="q_qa")
                    nc.scalar.activation(qa[:], yvi, AF.Sin, bias=negpi[:], scale=SIN_SCALE)
                    awt = big.tile([128, DC * QL], BF16, tag=f"aw{m}_{t}")
                    nc.vector.scalar_tensor_tensor(
                        awt[:], qa[:], cm, wcol_sb[:], op0=ALU.mult, op1=ALU.mult
                    )
                    aw[(m, t)] = awt

            # ---- main: K-side basis + scores matmuls ----
            scores_ps = ps_sc.tile([QL, L], F32, tag="scores")
            n_mm = 2 * M_HARM * DC
            idx = 0
            for m in range(M_HARM):
                s_turn = float(OMEGAS[m] / TWO_PI)
                for t, phase in ((0, 0.25), (1, 0.0)):  # K side: t=0 cos, t=1 sin
                    yk = yv_pool.tile([128, DC * L], F32, tag="k_yv")
                    nc.vector.tensor_scalar(
                        yk[:], kt_all[:], s_turn, MAGIC + phase, op0=ALU.mult, op1=ALU.add
                    )
                    yki = yk[:].bitcast(I32)
                    nc.vector.tensor_scalar(
                        yki, yki, (1 << NBITS) - 1, None, op0=ALU.bitwise_and
                    )
                    kb = kb_pool.tile([128, DC * L], BF16, tag="k_kb")
                    nc.scalar.activation(kb[:], yki, AF.Sin, bias=negpi[:], scale=SIN_SCALE)
                    for dc in range(DC):
                        nc.tensor.matmul(
                            scores_ps[:],
                            aw[(m, t)][:, dc * QL:(dc + 1) * QL],
                            kb[:, dc * L:(dc + 1) * L],
                            start=(idx == 0), stop=(idx == n_mm - 1),
                        )
                        idx += 1

            # ---- softmax over k (free axis) ----
            scores_sb = big.tile([QL, L], F32, tag="scores_sb")
            nc.vector.tensor_tensor(scores_sb[:], scores_ps[:], maskb_sb[:], op=ALU.add)
            negmx = big.tile([QL, 1], F32, tag="negmx")
            nc.vector.tensor_reduce(
                negmx[:], scores_sb[:], axis=mybir.AxisListType.X, op=ALU.max, negate=True
            )
            exp_sb = big.tile([QL, L], F32, tag="exp_sb")
            nc.scalar.activation(exp_sb[:], scores_sb[:], AF.Exp, bias=negmx[:])
            sm = big.tile([QL, 1], F32, tag="sm")
            nc.vector.tensor_reduce(sm[:], exp_sb[:], axis=mybir.AxisListType.X, op=ALU.add)
            rs = big.tile([QL, 1], F32, tag="rs")
            nc.vector.reciprocal(rs[:], sm[:])
            probs = big.tile([QL, L], BF16, tag="probs")
            nc.vector.tensor_scalar(probs[:], exp_sb[:], rs[:], None, op0=ALU.mult)

            # ---- probs^T via PE transpose (bf16) ----
            probsT_sb = []
            for kc in range(KC):
                ps = ps_sm.tile([128, QL], BF16, tag="ps_pt")
                nc.tensor.matmul(
                    ps[:], probs[:, kc * 128:(kc + 1) * 128], eye64_sb[:],
                    is_transpose=True,
                )
                pt = big.tile([128, QL], BF16, tag=f"pt{kc}")
                nc.vector.tensor_copy(pt[:], ps[:])
                probsT_sb.append(pt)

            # ---- weighted^T[do] = sum_kc hs[kc,:,do-slice]^T probsT[kc] (bf16) ----
            wT_sb = []
            for do in range(DC):
                ps = ps_sm.tile([128, QL], F32, tag="ps_sm")
                for kc in range(KC):
                    nc.tensor.matmul(
                        ps[:], hs_sb[kc][:, do * 128:(do + 1) * 128], probsT_sb[kc][:],
                        start=(kc == 0), stop=(kc == KC - 1),
                    )
                wt = big.tile([128, QL], BF16, tag=f"wt{do}")
                nc.vector.tensor_copy(wt[:], ps[:])
                wT_sb.append(wt)

            # ---- out = weighted @ Wt + (bt - bk) + (Q + bq + bk) ----
            out_sb = big.tile([QL, D], F32, tag="out_sb")
            H = D // 2
            for h in range(2):
                ps = ps_sm.tile([QL, H], F32, tag="ps_sm")
                for do in range(DC):
                    nc.tensor.matmul(
                        ps[:], wT_sb[do][:], Wt_sb[do][:, h * H:(h + 1) * H],
                        start=(do == 0), stop=False,
                    )
                nc.tensor.matmul(
                    ps[:], ones_sb[:], btk_sb[:, h * H:(h + 1) * H],
                    start=False, stop=False,
                )
                for j in range(3):
                    do = h * 3 + j
                    nc.tensor.matmul(
                        ps[:, j * 128:(j + 1) * 128],
                        qt_all[:, do * QL:(do + 1) * QL],
                        eye128_sb[:],
                        is_transpose=True,
                        start=False, stop=(j == 2),
                        skip_group_check=True,
                    )
                nc.vector.tensor_copy(out_sb[:, h * H:(h + 1) * H], ps[:])

            nc.sync.dma_start(out_dram[:], out_sb[:])

    nc.compile()
    return nc


def _get_nc():
    global _NC
    if _NC is None:
        _NC = _build()
    return _NC


def kernel(hidden_states, attention_mask, Wq, bq, Wk, bk, w_att, b_att, Wt, bt):
    nc = _get_nc()

    hs = np.ascontiguousarray(np.asarray(hidden_states, dtype=np.float32)[0])  # [L, D]
    Wq = np.asarray(Wq, dtype=np.float32)
    Wk = np.asarray(Wk, dtype=np.float32)
    Wt = np.asarray(Wt, dtype=np.float32)
    bq = np.asarray(bq, dtype=np.float32)
    bk = np.asarray(bk, dtype=np.float32)
    bt = np.asarray(bt, dtype=np.float32)
    w_att = np.asarray(w_att, dtype=np.float32)
    b_att = np.float32(np.asarray(b_att))
    mask = np.asarray(attention_mask, dtype=np.float32).reshape(-1)  # [L] (B=1)

    hsT = np.ascontiguousarray(hs.T)                                  # [D, L]
    common = {
        "Wq": Wq.reshape(DC, 128, D),
        "hsT": hsT.astype(NPBF16).reshape(DC, 128, L),
        "Wk": Wk.astype(NPBF16).reshape(DC, 128, D),
        "wcol": np.ascontiguousarray(np.repeat(w_att.reshape(DC, 128).T, QL, axis=1)),  # [128, DC*QL]
        "bqk": (bq + bk).reshape(1, D),
        "ones": np.ones((1, QL), np.float32),
        "hs": hs.astype(NPBF16).reshape(KC, 128, D),
        "Wt": Wt.astype(NPBF16).reshape(DC, 128, D),
        "eye64": np.eye(QL, dtype=NPBF16),
        "eye128": np.eye(128, dtype=np.float32),
        "maskb": np.ascontiguousarray(
            np.broadcast_to(mask + b_att, (QL, L)).astype(np.float32)
        ),
        "btk": (bt - bk).reshape(1, D),
    }
    in_maps = []
    for c in range(CORES):
        m = dict(common)
        m["hsTloc"] = np.ascontiguousarray(
            hsT[:, c * QL:(c + 1) * QL].reshape(DC, 128, QL)
        )
        in_maps.append(m)

    trace = bool(int(os.environ.get("BASSK_TRACE", "0")))
    res = run_bass_kernel_spmd(nc, in_maps, core_ids=list(range(CORES)), trace=trace)
    if trace:
        kernel.last_exec_time_ns = res.exec_time_ns
        kernel.last_results = res

    out = np.concatenate([res.results[c]["out"] for c in range(CORES)], axis=0)
    return out.reshape(B, L, D).astype(np.float32)


# revision 21
# speedup vs baseline: 1.1282x; 1.0477x over previous
"""Additive (Bahdanau) attention fused Trainium2 kernel.

Strategy
--------
The reference materializes a [B, Lq, Lk, D] = 768MB broadcast intermediate:
    scores[q,k] = sum_d w_d * tanh(Q[q,d] + K[k,d]) + b_att
We never materialize it.  tanh(q+k) is approximated by a truncated Fourier
sine series P(x) = sum_m c_m sin(omega_m x) fit on [-5.2, 5.2]; the angle
addition formula makes each term separable:
    sin(w(q+k)) = sin(wq)cos(wk) + cos(wq)sin(wk)
so scores = A @ B^T with A = [per-q sin/cos basis * c_m * w_d] (bf16) and
B = [per-k cos/sin basis] (bf16), contracting over (m, trig, d) = 2*M*768 on
the TensorEngine.  Basis tensors are built with a magic-number range
reduction on the VectorEngine (ACT's Sin is only valid on |x| <~ 3.2):
    tau = x * omega/2pi + (768.0 + phase_turns)   # fp32, ulp = 2^-14
    w14 = lowbits14(bitpattern(tau))              # frac(turns) * 16384
    basis = Sin(w14 * 2pi/16384 - pi)             # = -sin(omega x + phase)
The global -1 appears on BOTH sides of every product, so it cancels.

The final +Q output term reuses the already-computed Q^T (which carries
bq+bk) via accumulating PE transposes; the bias row compensates with
bt - bk.

Sharding: sequence-parallel over the query axis -- each of the 8 cores owns
L/8 = 64 queries; hidden_states / weights / K are replicated.  Per-core
output slab [64, 768] is concatenated on the host.
"""

import os
import sys

for _p in ("/opt/trn_rl_repo",):
    if _p not in sys.path:
        sys.path.insert(0, _p)

import numpy as np
import ml_dtypes

import concourse.bacc as bacc
import concourse.tile as tile
from concourse import mybir
from concourse.bass_utils import run_bass_kernel_spmd

AF = mybir.ActivationFunctionType
ALU = mybir.AluOpType
F32 = mybir.dt.float32
BF16 = mybir.dt.bfloat16
I32 = mybir.dt.int32
NPBF16 = ml_dtypes.bfloat16

B, L, D = 1, 512, 768
CORES = 8
QL = L // CORES          # 64 queries per core
DC = D // 128            # 6 chunks of 128 along d
KC = L // 128            # 4 chunks of 128 along k

M_HARM = 5
PERIOD = 6.0
FIT_RANGE = 5.2
TWO_PI = float(2 * np.pi)
MAGIC = 768.0            # 1.5 * 2^9 -> fp32 ulp 2^-14 for values near 768
NBITS = 14
SIN_SCALE = TWO_PI / (1 << NBITS)


def _fit_coefficients():
    om = np.pi * np.arange(1, M_HARM + 1) / PERIOD
    g = np.linspace(-FIT_RANGE, FIT_RANGE, 8001)
    A = np.sin(np.outer(g, om))
    coef, *_ = np.linalg.lstsq(A, np.tanh(g), rcond=None)
    return om.astype(np.float64), coef.astype(np.float64)

OMEGAS, COEFS = _fit_coefficients()

_NC = None


def _build():
    nc = bacc.Bacc("TRN2", target_bir_lowering=False, debug=False)

    dr = {}
    # critical-path inputs first (QT/KT + Q basis), bulk epilogue inputs last
    dr["hsTloc"] = nc.dram_tensor("hsTloc", [DC, 128, QL], F32, kind="ExternalInput")
    dr["Wq"] = nc.dram_tensor("Wq", [DC, 128, D], F32, kind="ExternalInput")
    dr["hsT"] = nc.dram_tensor("hsT", [DC, 128, L], BF16, kind="ExternalInput")
    dr["Wk"] = nc.dram_tensor("Wk", [DC, 128, D], BF16, kind="ExternalInput")
    dr["wcol"] = nc.dram_tensor("wcol", [128, DC * QL], F32, kind="ExternalInput")
    dr["bqk"] = nc.dram_tensor("bqk", [1, D], F32, kind="ExternalInput")
    dr["ones"] = nc.dram_tensor("ones", [1, QL], F32, kind="ExternalInput")
    dr["hs"] = nc.dram_tensor("hs", [KC, 128, D], BF16, kind="ExternalInput")
    dr["Wt"] = nc.dram_tensor("Wt", [DC, 128, D], BF16, kind="ExternalInput")
    dr["eye64"] = nc.dram_tensor("eye64", [QL, QL], BF16, kind="ExternalInput")
    dr["eye128"] = nc.dram_tensor("eye128", [128, 128], F32, kind="ExternalInput")
    dr["maskb"] = nc.dram_tensor("maskb", [QL, L], F32, kind="ExternalInput")
    dr["btk"] = nc.dram_tensor("btk", [1, D], F32, kind="ExternalInput")  # bt - bk
    out_dram = nc.dram_tensor("out", [QL, D], F32, kind="ExternalOutput")

    with tile.TileContext(nc) as tc:
        with (
            tc.tile_pool(name="big", bufs=1) as big,
            tc.tile_pool(name="qa", bufs=2) as qa_pool,
            tc.tile_pool(name="yv", bufs=3) as yv_pool,
            tc.tile_pool(name="kb", bufs=4) as kb_pool,
            tc.tile_pool(name="ps_sc", bufs=1, space="PSUM") as ps_sc,
            tc.tile_pool(name="ps_kt", bufs=2, space="PSUM") as ps_kt,
            tc.tile_pool(name="ps_sm", bufs=2, space="PSUM") as ps_sm,
        ):
            # ---- persistent SBUF tiles + input DMAs ----
            # issue split across three engines so descriptor-gen doesn't
            # serialize on one sequencer; critical path (QT/KT) first
            def load(shape, src_ap, tag, dt=F32, eng=None):
                t = big.tile(shape, dt, tag=tag)
                (eng or nc.sync).dma_start(t[:], src_ap)
                return t

            negpi = big.tile([128, 1], F32, tag="negpi")
            nc.gpsimd.memset(negpi[:], -float(np.pi))

            hsT_sb = [load([128, L], dr["hsT"][dc], f"hsT{dc}", BF16, nc.scalar) for dc in range(DC)]
            Wk_sb = [load([128, D], dr["Wk"][dc], f"Wk{dc}", BF16) for dc in range(DC)]
            hsTloc_sb = [load([128, QL], dr["hsTloc"][dc], f"hsTloc{dc}") for dc in range(DC)]
            Wq_sb = [load([128, D], dr["Wq"][dc], f"Wq{dc}") for dc in range(DC)]
            wcol_sb = load([128, DC * QL], dr["wcol"][:], "wcol")
            bqk_sb = load([1, D], dr["bqk"][:], "bqk")
            ones_sb = load([1, QL], dr["ones"][:], "ones")
            hs_sb = [load([128, D], dr["hs"][kc], f"hs{kc}", BF16) for kc in range(KC)]
            Wt_sb = [load([128, D], dr["Wt"][dc], f"Wt{dc}", BF16) for dc in range(DC)]
            eye64_sb = load([QL, QL], dr["eye64"][:], "eye64", BF16)
            eye128_sb = load([128, 128], dr["eye128"][:], "eye128")
            maskb_sb = load([QL, L], dr["maskb"][:], "maskb")
            btk_sb = load([1, D], dr["btk"][:], "btk")

            # ---- KT = Wk^T hsT (bf16 inputs, f32 accum), laid out [128, DC*L] ----
            kt_all = big.tile([128, DC * L], F32, tag="kt_all")
            for do in range(DC):
                ps = ps_kt.tile([128, L], F32, tag="ps_kt")
                for di in range(DC):
                    nc.tensor.matmul(
                        ps[:], Wk_sb[di][:, do * 128:(do + 1) * 128], hsT_sb[di][:],
                        start=(di == 0), stop=(di == DC - 1),
                    )
                nc.scalar.copy(kt_all[:, do * L:(do + 1) * L], ps[:])

            # ---- QT = (Wq^T hsT_loc) + (bq+bk), laid out [128, DC*QL] ----
            qt_all = big.tile([128, DC * QL], F32, tag="qt_all")
            for do in range(DC):
                ps = ps_sm.tile([128, QL], F32, tag="ps_sm")
                for di in range(DC):
                    nc.tensor.matmul(
                        ps[:], Wq_sb[di][:, do * 128:(do + 1) * 128], hsTloc_sb[di][:],
                        start=(di == 0), stop=False,
                    )
                nc.tensor.matmul(
                    ps[:], bqk_sb[:, do * 128:(do + 1) * 128], ones_sb[:],
                    start=False, stop=True,
                )
                nc.vector.tensor_copy(qt_all[:, do * QL:(do + 1) * QL], ps[:])

            # ---- Q-side basis: AwT[(m,t)] [128, DC*QL] bf16 ----
            # t=0: sin(om Q) pairs with K cos; t=1: cos(om Q) pairs with K sin
            aw = {}
            for m in range(M_HARM):
                s_turn = float(OMEGAS[m] / TWO_PI)
                cm = float(COEFS[m])
                for t, phase in ((0, 0.0), (1, 0.25)):
                    yv = qa_pool.tile([128, DC * QL], F32, tag="q_yv")
                    nc.vector.tensor_scalar(
                        yv[:], qt_all[:], s_turn, MAGIC + phase, op0=ALU.mult, op1=ALU.add
                    )
                    yvi = yv[:].bitcast(I32)
                    nc.vector.tensor_scalar(
                        yvi, yvi, (1 << NBITS) - 1, None, op0=ALU.bitwise_and
                    )
                    qa = qa_pool.tile([128, DC * QL], F32, tag="q_qa")
                    nc.scalar.activation(qa[:], yvi, AF.Sin, bias=negpi[:], scale=SIN_SCALE)
                    awt = big.tile([128, DC * QL], BF16, tag=f"aw{m}_{t}")
                    nc.vector.scalar_tensor_tensor(
                        awt[:], qa[:], cm, wcol_sb[:], op0=ALU.mult, op1=ALU.mult
                    )
                    aw[(m, t)] = awt

            # ---- main: K-side basis + scores matmuls ----
            scores_ps = ps_sc.tile([QL, L], F32, tag="scores")
            n_mm = 2 * M_HARM * DC
            idx = 0
            for m in range(M_HARM):
                s_turn = float(OMEGAS[m] / TWO_PI)
                for t, phase in ((0, 0.25), (1, 0.0)):  # K side: t=0 cos, t=1 sin
                    yk = yv_pool.tile([128, DC * L], F32, tag="k_yv")
                    nc.vector.tensor_scalar(
                        yk[:], kt_all[:], s_turn, MAGIC + phase, op0=ALU.mult, op1=ALU.add
                    )
                    yki = yk[:].bitcast(I32)
                    nc.vector.tensor_scalar(
                        yki, yki, (1 << NBITS) - 1, None, op0=ALU.bitwise_and
                    )
                    kb = kb_pool.tile([128, DC * L], BF16, tag="k_kb")
                    nc.scalar.activation(kb[:], yki, AF.Sin, bias=negpi[:], scale=SIN_SCALE)
                    for dc in range(DC):
                        nc.tensor.matmul(
                            scores_ps[:],
                            aw[(m, t)][:, dc * QL:(dc + 1) * QL],
                            kb[:, dc * L:(dc + 1) * L],
                            start=(idx == 0), stop=(idx == n_mm - 1),
                        )
                        idx += 1

            # ---- softmax over k (free axis) ----
            scores_sb = big.tile([QL, L], F32, tag="scores_sb")
            nc.vector.tensor_tensor(scores_sb[:], scores_ps[:], maskb_sb[:], op=ALU.add)
            negmx = big.tile([QL, 1], F32, tag="negmx")
            nc.vector.tensor_reduce(
                negmx[:], scores_sb[:], axis=mybir.AxisListType.X, op=ALU.max, negate=True
            )
            exp_sb = big.tile([QL, L], F32, tag="exp_sb")
            nc.scalar.activation(exp_sb[:], scores_sb[:], AF.Exp, bias=negmx[:])
            sm = big.tile([QL, 1], F32, tag="sm")
            nc.vector.tensor_reduce(sm[:], exp_sb[:], axis=mybir.AxisListType.X, op=ALU.add)
            rs = big.tile([QL, 1], F32, tag="rs")
            nc.vector.reciprocal(rs[:], sm[:])
            probs = big.tile([QL, L], BF16, tag="probs")
            nc.vector.tensor_scalar(probs[:], exp_sb[:], rs[:], None, op0=ALU.mult)

            # ---- probs^T via PE transpose (bf16) ----
            probsT_sb = []
            for kc in range(KC):
                ps = ps_sm.tile([128, QL], BF16, tag="ps_pt")
                nc.tensor.matmul(
                    ps[:], probs[:, kc * 128:(kc + 1) * 128], eye64_sb[:],
                    is_transpose=True,
                )
                pt = big.tile([128, QL], BF16, tag=f"pt{kc}")
                nc.scalar.copy(pt[:], ps[:])
                probsT_sb.append(pt)

            # ---- weighted^T[do] = sum_kc hs[kc,:,do-slice]^T probsT[kc] (bf16) ----
            wT_sb = []
            for do in range(DC):
                ps = ps_sm.tile([128, QL], F32, tag="ps_sm")
                for kc in range(KC):
                    nc.tensor.matmul(
                        ps[:], hs_sb[kc][:, do * 128:(do + 1) * 128], probsT_sb[kc][:],
                        start=(kc == 0), stop=(kc == KC - 1),
                    )
                wt = big.tile([128, QL], BF16, tag=f"wt{do}")
                nc.scalar.copy(wt[:], ps[:])
                wT_sb.append(wt)

            # ---- out = weighted @ Wt + (bt - bk) + (Q + bq + bk) ----
            out_sb = big.tile([QL, D], F32, tag="out_sb")
            H = D // 2
            for h in range(2):
                ps = ps_sm.tile([QL, H], F32, tag="ps_sm")
                for do in range(DC):
                    nc.tensor.matmul(
                        ps[:], wT_sb[do][:], Wt_sb[do][:, h * H:(h + 1) * H],
                        start=(do == 0), stop=False,
                    )
                nc.tensor.matmul(
                    ps[:], ones_sb[:], btk_sb[:, h * H:(h + 1) * H],
                    start=False, stop=False,
                )
                for j in range(3):
                    do = h * 3 + j
                    nc.tensor.matmul(
                        ps[:, j * 128:(j + 1) * 128],
                        qt_all[:, do * QL:(do + 1) * QL],
                        eye128_sb[:],
                        is_transpose=True,
                        start=False, stop=(j == 2),
                        skip_group_check=True,
                    )
                nc.vector.tensor_copy(out_sb[:, h * H:(h + 1) * H], ps[:])

            nc.sync.dma_start(out_dram[:], out_sb[:])

    nc.compile()
    return nc


def _get_nc():
    global _NC
    if _NC is None:
        _NC = _build()
    return _NC


def kernel(hidden_states, attention_mask, Wq, bq, Wk, bk, w_att, b_att, Wt, bt):
    nc = _get_nc()

    hs = np.ascontiguousarray(np.asarray(hidden_states, dtype=np.float32)[0])  # [L, D]
    Wq = np.asarray(Wq, dtype=np.float32)
    Wk = np.asarray(Wk, dtype=np.float32)
    Wt = np.asarray(Wt, dtype=np.float32)
    bq = np.asarray(bq, dtype=np.float32)
    bk = np.asarray(bk, dtype=np.float32)
    bt = np.asarray(bt, dtype=np.float32)
    w_att = np.asarray(w_att, dtype=np.float32)
    b_att = np.float32(np.asarray(b_att))
    mask = np.asarray(attention_mask, dtype=np.float32).reshape(-1)  # [L] (B=1)

    hsT = np.ascontiguousarray(hs.T)                                  # [D, L]
    common = {
        "Wq": Wq.reshape(DC, 128, D),
        "hsT": hsT.astype(NPBF16).reshape(DC, 128, L),
        "Wk": Wk.astype(NPBF16).reshape(DC, 128, D),
        "wcol": np.ascontiguousarray(np.repeat(w_att.reshape(DC, 128).T, QL, axis=1)),  # [128, DC*QL]
        "bqk": (bq + bk).reshape(1, D),
        "ones": np.ones((1, QL), np.float32),
        "hs": hs.astype(NPBF16).reshape(KC, 128, D),
        "Wt": Wt.astype(NPBF16).reshape(DC, 128, D),
        "eye64": np.eye(QL, dtype=NPBF16),
        "eye128": np.eye(128, dtype=np.float32),
        "maskb": np.ascontiguousarray(
            np.broadcast_to(mask + b_att, (QL, L)).astype(np.float32)
        ),
        "btk": (bt - bk).reshape(1, D),
    }
    in_maps = []
    for c in range(CORES):
        m = dict(common)
        m["hsTloc"] = np.ascontiguousarray(
            hsT[:, c * QL:(c + 1) * QL].reshape(DC, 128, QL)
        )
        in_maps.append(m)

    trace = bool(int(os.environ.get("BASSK_TRACE", "0")))
    res = run_bass_kernel_spmd(nc, in_maps, core_ids=list(range(CORES)), trace=trace)
    if trace:
        kernel.last_exec_time_ns = res.exec_time_ns
        kernel.last_results = res

    out = np.concatenate([res.results[c]["out"] for c in range(CORES)], axis=0)
    return out.reshape(B, L, D).astype(np.float32)
